# revision 46
# baseline (speedup 1.0000x reference)
"""Trainium2 Bass/Tile kernel for the InterPart block (nn_InterPart_45827301048588).

Contract: kernel(**inputs) takes the FULL numpy inputs of the reference
(x [32,256,256,25] f32 plus weights) and returns the FULL [32,256,256,25]
f32 output. Data-parallel over N across 8 NeuronCores; BN batch stats are
synchronized with a tiny in-kernel AllReduce.

Key design (mathematically exact up to bf16 rounding; tolerance is 2e-2,
measured error ~3e-3):
  - Everything runs in bf16 on-device. x is converted to bf16 HOST-side so
    the HBM read is 13.1 MB/core instead of 26.2; the output is written
    bf16 and converted back to f32 host-side (writes halved too).
  - x is read from HBM ONCE (chunked prefetch) and stays resident in SBUF
    for the pass-2 residual.
  - bg/bw biases cancel exactly through BN (softmax rows sum to 1 and
    per-channel constants are absorbed by the mean subtraction) -> dropped.
  - 1/V of the V-mean is folded into Wth/Wph host-side.
  - BN stats (mean/var of r = Ww@y over (batch, T)) come from a Gram
    matrix: G = sum_{n,v,t'} y y^T plus s = sum y (a ones-column riding the
    same PSUM accumulation). Then S1 = Ww s and S2[c] = Ww[c,:] G Ww[c,:]^T
    exactly, which removes both the pass-1 Ww@y GEMM and the bn_stats scan.
    stats_mode="bn" keeps the direct bn_stats path as a fallback.
  - Pass 2 recomputes r = Ww@y from SBUF-resident y (no HBM traffic) and
    applies z = a*r + d2 + x with a = gamma*rsqrt(var+eps),
    d2 = beta - a*mean, with the elementwise work split across the
    Act/DVE/Pool engines. PSUM pools are scoped: pass-1 pools are released
    so pass 2 gets a 4-deep PSUM pipeline (8 banks).
"""

import sys
from contextlib import ExitStack

import numpy as np
import ml_dtypes

if "/opt/trn_rl_repo" not in sys.path:
    sys.path.insert(0, "/opt/trn_rl_repo")

N, C, T, V = 32, 256, 256, 25
CI = 128
P = 128
EPS = 1e-5
NCORES = 8
NPC = N // NCORES          # batches per core
NV = NPC * V               # (n,v) units per core
BT_GLOBAL = float(N * V * T)
CNT_LOCAL = float(NPC * V * T)
BF16 = ml_dtypes.bfloat16

_CACHE = {}

# 2v batches covering V=25
VB = [(2 * i, 2) for i in range(12)] + [(24, 1)]


def _build_nc(stats_mode="gram", n_replicas=NCORES, collective=True):
    import concourse.bass as bass  # noqa: F401
    import concourse.mybir as mybir
    import concourse.tile as tile
    from concourse import bacc
    from concourse.masks import make_identity

    f32 = mybir.dt.float32
    bf16 = mybir.dt.bfloat16
    AF = mybir.ActivationFunctionType
    ALU = mybir.AluOpType

    gram = stats_mode == "gram"

    nc = bacc.Bacc("TRN2", target_bir_lowering=False, debug=False,
                   num_devices=n_replicas)

    # DRAM I/O (per core shapes); c = ch*128 + c_lo everywhere
    # wpk packs [wg, wth, wph, ww] (ww in [ci, ch, c_lo] orientation);
    # fpk packs f32 vectors: [bth, bph, gamma0, gamma1, beta0, beta1].
    x = nc.dram_tensor("x", [NPC, 2, P, V, T], bf16, kind="ExternalInput")
    wpk = nc.dram_tensor("wpk", [P, 5, 2, CI], bf16, kind="ExternalInput")
    fpk = nc.dram_tensor("fpk", [P, 6], f32, kind="ExternalInput")
    out = nc.dram_tensor("out", [NPC, 2, P, V, T], bf16, kind="ExternalOutput")

    def prefetch_x(n):
        # chunked so (a) n=0 compute starts after the first v's arrive and
        # (b) the yT dma-transposes can slot between chunks on the DMA bus
        for c0, c1 in ((0, 8), (8, 16), (16, V)):
            nc.sync.dma_start(
                xres[:, n, :, c0:c1, :],
                x[n, :, :, c0:c1, :].rearrange("c p v t -> p c v t"))

    with tile.TileContext(nc) as tc, ExitStack() as st:
        constp = st.enter_context(tc.tile_pool(name="const", bufs=1))
        bigp = st.enter_context(tc.tile_pool(name="big", bufs=1))
        gtp = st.enter_context(tc.tile_pool(name="gtp", bufs=1))
        work = st.enter_context(tc.tile_pool(name="work", bufs=2))
        outst = st.enter_context(tc.tile_pool(name="outst", bufs=3))
        small = st.enter_context(tc.tile_pool(name="small", bufs=4))
        dramp = st.enter_context(
            tc.tile_pool(name="dram", bufs=1, space="DRAM"))

        # ---- constants (2 DMAs total) ----
        wpk_sb = constp.tile([P, 5, 2, CI], bf16)
        nc.sync.dma_start(wpk_sb[:], wpk[:])
        fpk_sb = constp.tile([P, 6], f32)
        nc.sync.dma_start(fpk_sb[:], fpk[:])
        wg_sb = wpk_sb[:, 0]
        wth_sb = wpk_sb[:, 1]
        wph_sb = wpk_sb[:, 2]
        ww_sb = wpk_sb[:, 3]          # [ci, ch, c_lo]
        wwt_sb = wpk_sb[:, 4]         # [c_lo, ch, ci]
        bth_sb = fpk_sb[:, 0:1]
        bph_sb = fpk_sb[:, 1:2]
        ident = constp.tile([P, P], bf16)
        make_identity(nc, ident[:])
        ones_bf = constp.tile([P, 1], bf16)
        nc.vector.memset(ones_bf[:], 1.0)
        eps_sb = constp.tile([P, 1], f32)
        nc.vector.memset(eps_sb[:], EPS)

        # ---- big persistent buffers ----
        xres = bigp.tile([P, NPC, 2, V, T], bf16)   # resident x (13.1MB)
        ys = bigp.tile([CI, NV, T], bf16)           # y (6.55MB)
        if not gram:
            statb = bigp.tile([P, 2, len(VB) * NPC * 4 * 6], f32)

        prefetch_x(0)

        sloc = small.tile([P, 4], f32, tag="sloc")

        def copy_eng(k, dst, src):
            # PSUM sources: only Act/DVE may read PSUM (GPSIMD cannot)
            if k % 2 == 0:
                nc.scalar.copy(dst, src)
            else:
                nc.vector.tensor_copy(dst, src)

        # ============ scope A: pass 1 + stats (8 PSUM banks) ============
        with ExitStack() as stA:
            psA = stA.enter_context(
                tc.tile_pool(name="psA", bufs=3, space="PSUM"))
            psW = stA.enter_context(
                tc.tile_pool(name="psW", bufs=2, space="PSUM"))
            psT = stA.enter_context(
                tc.tile_pool(name="psT", bufs=(2 if gram else 1),
                             space="PSUM"))
            if gram:
                psG = stA.enter_context(
                    tc.tile_pool(name="psG", bufs=1, space="PSUM"))
                psG_t = psG.tile([CI, CI + 1], f32, tag="G")

            def tree_ops(nn):
                # xsum = sum_v x[:, nn, :, v, :] as a list of DVE thunks
                # (chunk-aligned for the n=0 prefetch; hoisted into the
                # previous n's y-loop otherwise so thph never waits).
                scr = work.tile([P, 2, 8, T], bf16, tag="tree", bufs=2)
                xs = work.tile([P, 2, T], bf16, tag="xsum", bufs=2)
                xn = xres[:, nn]
                return xs, [
                    lambda: nc.vector.tensor_add(
                        scr[:, :, 0:4, :], xn[:, :, 0:4, :],
                        xn[:, :, 4:8, :]),
                    lambda: nc.vector.tensor_add(
                        scr[:, :, 4:8, :], xn[:, :, 8:12, :],
                        xn[:, :, 12:16, :]),
                    lambda: nc.vector.tensor_add(
                        scr[:, :, 0:4, :], scr[:, :, 0:4, :],
                        scr[:, :, 4:8, :]),
                    lambda: nc.vector.tensor_add(
                        scr[:, :, 4:8, :], xn[:, :, 16:20, :],
                        xn[:, :, 20:24, :]),
                    lambda: nc.vector.tensor_add(
                        scr[:, :, 0:4, :], scr[:, :, 0:4, :],
                        scr[:, :, 4:8, :]),
                    lambda: nc.vector.tensor_add(
                        scr[:, :, 0:2, :], scr[:, :, 0:2, :],
                        scr[:, :, 2:4, :]),
                    lambda: nc.vector.tensor_add(
                        scr[:, :, 0, :], scr[:, :, 0, :], scr[:, :, 1, :]),
                    lambda: nc.vector.tensor_add(
                        xs[:], scr[:, :, 0, :], xn[:, :, 24, :]),
                ]

            xsum0, thunks0 = tree_ops(0)
            for t in thunks0:
                t()
            xsum_next = xsum0

            for n in range(NPC):
                if n + 1 < NPC:
                    prefetch_x(n + 1)
                xsum = xsum_next
                tree_next = None

                gt_n = gtp.tile([P, V, 2, CI], bf16, tag="gt")

                def g_batch(b):
                    v0, bs = VB[b]
                    gps = psW.tile([P, 2, 2, CI], f32, tag="psW")
                    for q in range(bs):
                        for th in range(2):
                            for ch in range(2):
                                nc.tensor.matmul(
                                    gps[:, q, th, :],
                                    xres[:, n, ch, v0 + q,
                                         th * P:(th + 1) * P],
                                    wg_sb[:, ch, :],
                                    start=(ch == 0), stop=(ch == 1))
                    copy_eng(b, gt_n[:, v0:v0 + bs, :, :],
                             gps[:, 0:bs, :, :])

                # -- theta/phi + softmax interleaved with g batches so the
                # PE has g work while Act/DVE run the softmax chain.
                th_sb = work.tile([CI, T], bf16, tag="th", bufs=1)
                ph_sb = work.tile([CI, T], bf16, tag="ph", bufs=1)
                for w_sb, b_sb, dst in ((wth_sb, bth_sb, th_sb),
                                        (wph_sb, bph_sb, ph_sb)):
                    ps = psA.tile([CI, T], f32, tag="psA")
                    for ch in range(2):
                        nc.tensor.matmul(ps[:], w_sb[:, ch, :],
                                         xsum[:, ch, :],
                                         start=(ch == 0), stop=(ch == 1))
                    nc.scalar.activation(dst[:], ps[:], AF.Identity,
                                         bias=b_sb[:], scale=1.0)

                g_batch(0)
                g_batch(1)

                # logits are O(1) (sigma ~ 0.5): exp without max-subtraction
                fss = []
                for t1 in range(2):
                    fps = psA.tile([P, T], f32, tag="psA")
                    nc.tensor.matmul(fps[:], th_sb[:, t1 * P:(t1 + 1) * P],
                                     ph_sb[:], start=True, stop=True)
                    fs = work.tile([P, T], bf16, tag="fs", bufs=2)
                    ssum = small.tile([P, 1], f32, tag="ssum")
                    nc.scalar.activation(fs[:], fps[:], AF.Exp, bias=0.0,
                                         scale=1.0, accum_out=ssum[:])
                    rec = small.tile([P, 1], f32, tag="rec")
                    nc.vector.reciprocal(rec[:], ssum[:])
                    nc.vector.tensor_scalar_mul(fs[:], fs[:], rec[:])
                    fss.append(fs)

                g_batch(2)
                g_batch(3)

                fT = work.tile([P, 2, T], bf16, tag="fT", bufs=1)
                for t1 in range(2):
                    tpf = psT.tile([P, 2, P], bf16, tag="psT")
                    for t2 in range(2):
                        nc.tensor.transpose(
                            tpf[:, t2, :], fss[t1][:, t2 * P:(t2 + 1) * P],
                            ident[:])
                    nc.scalar.copy(fT[:, :, t1 * P:(t1 + 1) * P], tpf[:])

                for b in range(4, len(VB)):
                    g_batch(b)

                # -- y in 2v batches (+ stats feed two batches behind).
                # yT comes from a DMA xbar transpose (SBUF->SBUF, chunked
                # block layout): no PE transposes, no engine copies.
                def stats_tail(v0, bs, treeb):
                    idx = n * V + v0
                    yt = work.tile([P, 4, CI], bf16, tag="yt", bufs=3)
                    nc.sync.dma_start_transpose(
                        yt[:, 0:2 * bs, :], ys[:, idx:idx + bs, :])
                    first = (n == 0 and v0 == 0)
                    last = (n == NPC - 1 and v0 + bs == V)
                    for j in range(2 * bs):
                        nc.tensor.matmul(
                            psG_t[:, 0:CI], yt[:, j, :], yt[:, j, :],
                            start=(first and j == 0),
                            stop=(last and j == 2 * bs - 1),
                            skip_group_check=True)
                        nc.tensor.matmul(
                            psG_t[:, CI:CI + 1], yt[:, j, :], ones_bf[:],
                            start=(first and j == 0),
                            stop=(last and j == 2 * bs - 1),
                            skip_group_check=True)

                for b, (v0, bs) in enumerate(VB):
                    yps = psA.tile([CI, 2, T], f32, tag="psA")
                    for q in range(bs):
                        for th in range(2):
                            nc.tensor.matmul(yps[:, q, :],
                                             gt_n[:, v0 + q, th, :],
                                             fT[:, th, :],
                                             start=(th == 0), stop=(th == 1))
                    copy_eng(b + 1, ys[:, n * V + v0:n * V + v0 + bs, :],
                             yps[:, 0:bs, :])
                    # hoisted xsum tree for n+1 (one DVE op per batch)
                    treeb = n + 1 < NPC and 2 <= b < 10
                    if treeb:
                        if tree_next is None:
                            xsum_next, tree_next = tree_ops(n + 1)
                        tree_next[b - 2]()
                    if not gram:
                        wyps = psW.tile([P, 2, 2, T // 2], f32, tag="psWb",
                                        bufs=2)
                        # bn fallback keeps the old layout: [P,2,2,T] needs
                        # 2 banks; use half-T tiles twice instead
                        for q in range(bs):
                            for ch in range(2):
                                for tt in range(2):
                                    nc.tensor.matmul(
                                        wyps[:, q, ch, :], ww_sb[:, ch, :],
                                        ys[:, n * V + v0 + q,
                                           tt * (T // 2):(tt + 1) * (T // 2)],
                                        start=True, stop=True)
                                    g6 = ((n * len(VB) + b) * 4
                                          + q * 2 + tt) * 6
                                    nc.vector.bn_stats(
                                        statb[:, ch, g6:g6 + 6],
                                        wyps[:, q, ch, :])
                    elif b >= 2:
                        stats_tail(*VB[b - 2], treeb)
                if gram:
                    stats_tail(*VB[-2], False)
                    stats_tail(*VB[-1], False)

            # ============ STATS -> (sum, sumsq) ============
            if gram:
                # G_sb = [G | s] bf16; S1 = Ww s ; S2 = diag(Ww G Ww^T):
                # K = G @ Ww^T (G symmetric), M2 = K .* Ww, S2 = colsum(M2)
                g_sb = work.tile([CI, CI + 1], bf16, tag="gsb", bufs=1)
                nc.scalar.copy(g_sb[:], psG_t[:])
                kps = psA.tile([CI, 2, P], f32, tag="psA")
                nc.tensor.matmul(kps[:], g_sb[:, 0:CI], ww_sb[:],
                                 start=True, stop=True)
                m2 = work.tile([CI, 2, P], bf16, tag="m2", bufs=1)
                nc.vector.tensor_mul(m2[:], kps[:], ww_sb[:])
                s1ps = psT.tile([P, 2, P], f32, tag="psT")
                for ch in range(2):
                    nc.tensor.matmul(s1ps[:, 0, ch:ch + 1], ww_sb[:, ch, :],
                                     g_sb[:, CI:CI + 1],
                                     start=True, stop=True)
                    nc.tensor.matmul(s1ps[:, 1, ch:ch + 1], m2[:, ch, :],
                                     ones_bf[:], start=True, stop=True)
                # sloc layout: [S1c0, S1c1, S2c0, S2c1]
                nc.scalar.copy(sloc[:, 0:2], s1ps[:, 0, 0:2])
                nc.scalar.copy(sloc[:, 2:4], s1ps[:, 1, 0:2])
            else:
                mv = small.tile([P, 2, 2], f32, tag="mv")
                for ch in range(2):
                    nc.vector.bn_aggr(mv[:, ch, :], statb[:, ch, :])
                tmp = small.tile([P, 1], f32, tag="tmpm")
                for ch in range(2):
                    m = mv[:, ch, 0:1]
                    var = mv[:, ch, 1:2]
                    nc.vector.tensor_scalar_mul(sloc[:, ch:ch + 1], m,
                                                CNT_LOCAL)
                    nc.vector.tensor_mul(tmp[:], m, m)
                    nc.vector.tensor_add(tmp[:], tmp[:], var)
                    nc.vector.tensor_scalar_mul(
                        sloc[:, 2 + ch:3 + ch], tmp[:], CNT_LOCAL)

        # ============ COLLECTIVE ============
        cin = dramp.tile([P, 4], f32)
        cout = dramp.tile([P, 4], f32)
        nc.sync.dma_start(cin[:], sloc[:])
        if collective:
            nc.gpsimd.collective_compute(
                "AllReduce", ALU.add,
                replica_groups=[list(range(n_replicas))],
                ins=[cin.opt()], outs=[cout.opt()])
        else:  # timeline-sim variant: plain copy stands in for AllReduce
            nc.sync.dma_start(cout[:], cin[:])
        sg = small.tile([P, 4], f32, tag="sg")
        nc.sync.dma_start(sg[:], cout[:])

        # a = gamma * rsqrt(var + eps); d2 = beta - a * mean  (both ch at
        # once: sg is [S1c0, S1c1, S2c0, S2c1], fpk has gammas/betas
        # adjacent)
        amat = small.tile([P, 2], f32, tag="amat")
        dmat = small.tile([P, 2], f32, tag="dmat")
        mr = small.tile([P, 2], f32, tag="mr")
        sd = small.tile([P, 2], f32, tag="sd")
        tmp2 = small.tile([P, 2], f32, tag="tmp2")
        nc.vector.tensor_scalar_mul(mr[:], sg[:, 0:2], 1.0 / BT_GLOBAL)
        nc.vector.tensor_scalar_mul(sd[:], sg[:, 2:4], 1.0 / BT_GLOBAL)
        nc.vector.tensor_mul(tmp2[:], mr[:], mr[:])
        nc.vector.tensor_sub(sd[:], sd[:], tmp2[:])
        nc.scalar.activation(sd[:], sd[:], AF.Sqrt, bias=eps_sb[:],
                             scale=1.0)
        nc.vector.reciprocal(sd[:], sd[:])
        nc.vector.tensor_mul(amat[:], sd[:], fpk_sb[:, 2:4])
        nc.vector.tensor_mul(tmp2[:], amat[:], mr[:])
        nc.vector.tensor_sub(dmat[:], fpk_sb[:, 4:6], tmp2[:])

        # fold a into the pass-2 weights: ww2 = a[c] * Ww, prepared from the
        # [c_lo, ch, ci]-oriented copy then DMA-transposed back to
        # [ci, ch, c_lo] for the GEMM.
        ww2t = work.tile([P, 2, CI], bf16, tag="ww2t", bufs=1)
        for ch in range(2):
            nc.vector.tensor_scalar_mul(ww2t[:, ch, :], wwt_sb[:, ch, :],
                                        amat[:, ch:ch + 1])
        ww2_sb = work.tile([CI, 2, P], bf16, tag="ww2", bufs=1)
        nc.sync.dma_start_transpose(ww2_sb[:], ww2t[:])

        # ============ scope B: PASS 2 (8 PSUM banks, depth 4) ============
        # z = a*(Ww@y) + d2 + x per 2v batch; scale+bias path cycled over
        # Act (60%) / DVE (20%) / Pool (20%); +x add on DVE (Pool for the
        # Pool-path groups). Output DMA'd per batch.
        with tc.tile_pool(name="psZ", bufs=4, space="PSUM") as psZ:
            GV = 5
            pend = None  # delayed (+x add, dma) of the previous sub-batch
            grp = 0
            for n in range(NPC):
                for g in range(V // GV):
                    stg = outst.tile([P, 2, GV, T], bf16, tag="stg")
                    for (q0, bs) in ((0, 2), (2, 2), (4, 1)):
                        v0 = g * GV + q0
                        idx = n * V + v0
                        zps = psZ.tile([P, 2, 2, T], f32, tag="psZ")
                        # the first groups use the UNSCALED weights so
                        # their GEMMs can run during the stats collective
                        # (ww2 = a*Ww depends on it); a is applied in their
                        # combine instead.
                        early = grp < 6
                        wsel = ww_sb if early else ww2_sb
                        for q in range(bs):
                            for ch in range(2):
                                nc.tensor.matmul(zps[:, q, ch, :],
                                                 wsel[:, ch, :],
                                                 ys[:, idx + q, :],
                                                 start=True, stop=True)
                        # combine = a*zps + d2 + x (early) / zps + d2 + x.
                        # Paths (GPSIMD cannot read PSUM):
                        # A = Act (bias|scale+bias) act, deferred DVE add x
                        # B = DVE STT (zps + d2 + x); TS+add when early
                        # C = Act act, deferred Pool add x
                        path = (0, 1, 2)[grp % 3]
                        grp += 1
                        for ch in range(2):
                            src = zps[:, 0:bs, ch, :]
                            dst = stg[:, ch, q0:q0 + bs, :]
                            if path == 1 and not early:
                                nc.vector.scalar_tensor_tensor(
                                    dst, src, dmat[:, ch:ch + 1],
                                    xres[:, n, ch, v0:v0 + bs, :],
                                    ALU.add, ALU.add)
                            elif path == 1:
                                nc.vector.tensor_scalar(
                                    dst, src, amat[:, ch:ch + 1],
                                    dmat[:, ch:ch + 1], ALU.mult, ALU.add)
                            else:
                                nc.scalar.activation(
                                    dst, src, AF.Identity,
                                    bias=dmat[:, ch:ch + 1],
                                    scale=(amat[:, ch:ch + 1] if early
                                           else 1.0))
                        if pend is not None:
                            pend()
                        sl = stg[:, :, q0:q0 + bs, :]
                        xsl = xres[:, n, :, v0:v0 + bs, :]
                        dma = (out[n, :, :, g * GV:(g + 1) * GV, :]
                               .rearrange("c p v t -> p c v t"),
                               stg) if q0 == 4 else None

                        def mk(sl=sl, xsl=xsl, path=path, dma=dma):
                            if path != 1:
                                eng = (nc.gpsimd if path == 2
                                       else nc.vector)
                                eng.tensor_add(sl, sl, xsl)
                            if dma is not None:
                                nc.sync.dma_start(dma[0], dma[1][:])
                        pend = mk
                    # one sub-batch of delay is enough; flush per stg is
                    # handled by the dma attached to the q0==4 sub-batch
            if pend is not None:
                pend()

    nc.compile()
    return nc


def _get_nc(stats_mode="gram", n_replicas=NCORES, collective=True):
    key = (stats_mode, n_replicas, collective)
    if key not in _CACHE:
        _CACHE[key] = _build_nc(stats_mode, n_replicas, collective)
    return _CACHE[key]


def prep_inputs(x, Wg, bg, Wth, bth, Wph, bph, Ww, bw, gamma, beta):
    """Host-side input prep -> list of per-core input dicts (bf16 upload)."""
    x = np.asarray(x, dtype=np.float32)
    # [N, C, T, V] -> [N, 2, P, V, T] bf16
    xt = x.reshape(N, 2, P, T, V).transpose(0, 1, 2, 4, 3)
    xt = np.ascontiguousarray(xt).astype(BF16)

    def ctile_lo(w):  # [C, CI] -> [c_lo, ch, CI] bf16
        return np.ascontiguousarray(
            np.asarray(w, np.float32).reshape(2, P, CI).transpose(1, 0, 2)
        ).astype(BF16)

    wg_h = ctile_lo(np.asarray(Wg, np.float32).T)
    wth_h = ctile_lo(np.asarray(Wth, np.float32).T / V)
    wph_h = ctile_lo(np.asarray(Wph, np.float32).T / V)
    ww_h = np.asarray(Ww, np.float32).T.reshape(CI, 2, P).astype(BF16)
    wwt_h = ctile_lo(np.asarray(Ww, np.float32))  # [c_lo, ch, ci]
    wpk_h = np.ascontiguousarray(
        np.stack([wg_h, wth_h, wph_h, ww_h, wwt_h], axis=1))  # [P,5,2,CI]
    fpk_h = np.ascontiguousarray(np.stack(
        [np.asarray(bth, np.float32),
         np.asarray(bph, np.float32),
         np.asarray(gamma, np.float32).reshape(2, P)[0],
         np.asarray(gamma, np.float32).reshape(2, P)[1],
         np.asarray(beta, np.float32).reshape(2, P)[0],
         np.asarray(beta, np.float32).reshape(2, P)[1]], axis=1))  # [P, 6]

    in_maps = []
    for c in range(NCORES):
        in_maps.append({
            "x": np.ascontiguousarray(xt[c * NPC:(c + 1) * NPC]),
            "wpk": wpk_h, "fpk": fpk_h,
        })
    return in_maps


def assemble_output(results):
    """Per-core 'out' [NPC, 2, P, V, T] bf16 -> full [N, C, T, V] f32."""
    parts = [np.asarray(r["out"]) for r in results]
    full = np.concatenate(parts, axis=0)              # [N, 2, P, V, T]
    full = full.astype(np.float32).reshape(N, C, V, T)
    return np.ascontiguousarray(full.transpose(0, 1, 3, 2))  # [N, C, T, V]


def kernel(x, Wg, bg, Wth, bth, Wph, bph, Ww, bw, gamma, beta,
           _trace=False, _stats_mode="gram"):
    from concourse.bass_utils import run_bass_kernel_spmd

    nc = _get_nc(_stats_mode)
    in_maps = prep_inputs(x, Wg, bg, Wth, bth, Wph, bph, Ww, bw, gamma, beta)
    try:
        res = run_bass_kernel_spmd(nc, in_maps, list(range(NCORES)),
                                   trace=_trace)
    except ModuleNotFoundError:
        res = run_bass_kernel_spmd(nc, in_maps, list(range(NCORES)),
                                   trace=False)
    out = assemble_output(res.results)
    kernel.last_results = res
    return out


if __name__ == "__main__":
    rng = np.random.default_rng(0)
    ins = {
        "x": rng.standard_normal((N, C, T, V), dtype=np.float32),
        "Wg": rng.standard_normal((CI, C), dtype=np.float32) / 16,
        "bg": rng.standard_normal(CI).astype(np.float32) / 16,
        "Wth": rng.standard_normal((CI, C)).astype(np.float32) / 16,
        "bth": rng.standard_normal(CI).astype(np.float32) / 16,
        "Wph": rng.standard_normal((CI, C)).astype(np.float32) / 16,
        "bph": rng.standard_normal(CI).astype(np.float32) / 16,
        "Ww": rng.standard_normal((C, CI)).astype(np.float32) / 11,
        "bw": rng.standard_normal(C).astype(np.float32) / 11,
        "gamma": rng.standard_normal(C).astype(np.float32) * 0.1,
        "beta": rng.standard_normal(C).astype(np.float32) * 0.1,
    }
    out = kernel(**ins)
    print("kernel ran, out shape:", out.shape)
